# revision 1
# baseline (speedup 1.0000x reference)
"""Chamfer distance L2 (mean-compressed) on 8 Trainium2 NeuronCores.

Sharding: data-parallel over batch B=16 -> 2 batches per core; each core
computes its own batches' min-distances; the host averages the per-core
partials (the "all-reduce" of the mean).

Per batch on one core the kernel computes the negated squared-distance matrix
    -d[i, j] = 2 p_i . g_j - |p_i|^2 - |g_j|^2
on the tensor engine.  To get fp32-grade accuracy at full bf16 PE rate
(1 cycle/row; plain fp32 matmuls are 4x slower and float32r is a low-precision
single-pass mode), every fp32 operand is split into 3 bf16 levels
(x = x0 + x1 + x2, residual ~2^-27) and the K=5 augmented-point contraction is
expanded into K=24 bf16 rows covering all product pairs down to 2^-27:
    2 p.g   : per dim, pairs (0,0),(0,1),(1,0),(1,1),(0,2),(2,0)  -> 18 rows
    -|p|^2  : 3 levels of p^2 against constant -1                 -> 3 rows
    -|g|^2  : constant 1 against 3 levels of -g^2                 -> 3 rows
PSUM accumulates in fp32, so the result matches an fp32 computation to ~1e-6.

Reductions (as max of -d), 128 pred points x 2048 gt per PSUM group:
  dist1 (min over gt):   VectorE reduce_max straight from PSUM
  dist2 (min over pred): elementwise max-accumulate into per-gt-column tiles
                         (nc.any -> lands on ScalarE, concurrent with VectorE),
                         folded across partitions with
                         gpsimd.partition_all_reduce(max)
"""

import numpy as np

_B, _N, _M = 16, 4096, 4096
_NCORES = 8
_BPC = _B // _NCORES  # batches per core
_PT = _N // 128       # pred tiles per batch
_HALF = 2048          # gt columns per psum group (4 banks)
_K = 24               # split-contraction depth
_NEG = -3.0e38

_cache = None


def _build_nc(reps=1, par=True, act_copy_on=True, mins_on=True):
    import concourse.mybir as mybir
    from concourse import tile, bass_isa, bacc

    dt = mybir.dt
    Alu = mybir.AluOpType
    f32, bf16, f16 = dt.float32, dt.bfloat16, dt.float16
    X = mybir.AxisListType.X

    nc = bacc.Bacc("TRN2", target_bir_lowering=False, debug=False)

    def act_copy(out, in_):
        # Plain copy pinned on ScalarE (walrus rejects TensorTensor /
        # TensorScalar / TensorReduce on Activation for TRN2, but TensorCopy
        # is fine).  Used to evacuate PSUM so VectorE+GpSimd can both chew
        # on the data from SBUF.
        eng = nc.scalar
        return eng.add_instruction(
            mybir.InstTensorCopy(
                name=f"I-{nc.next_id()}",
                ins=[eng.lower_ap(in_)],
                outs=[eng.lower_ap(out)],
            )
        )

    predA = nc.dram_tensor("predA", [_K, _BPC * _N], bf16, kind="ExternalInput").ap()
    gtA = nc.dram_tensor("gtA", [_K, _BPC * _M], bf16, kind="ExternalInput").ap()
    rowm_d = nc.dram_tensor(
        "rowmins", [128, _BPC * _PT], f32, kind="ExternalOutput"
    ).ap()
    if par:
        colm_d = nc.dram_tensor(
            "colmins", [_BPC * (_M // _HALF), _HALF], f32, kind="ExternalOutput"
        ).ap()
    else:
        colm_d = nc.dram_tensor(
            "colmins", [_BPC * (_M // _HALF) * 128, _HALF], f16,
            kind="ExternalOutput",
        ).ap()

    with tile.TileContext(nc) as tc:
        with (
            tc.tile_pool(name="io", bufs=1) as io,
            tc.tile_pool(name="dcp", bufs=3) as dcp,
            tc.tile_pool(name="acc", bufs=1) as acc,
            tc.tile_pool(name="work", bufs=2) as work,
            tc.tile_pool(name="ps", bufs=1, space="PSUM") as ps,
        ):
            pa = io.tile([_K, _BPC * _N], bf16, tag="pa")
            ga = io.tile([_K, _BPC * _M], bf16, tag="ga")
            nc.sync.dma_start(pa[:], predA[:])
            nc.sync.dma_start(ga[:], gtA[:])
            rowm = io.tile([128, _BPC * _PT], f32, tag="rowm")

            if not mins_on:
                nc.gpsimd.memset(rowm[:], 0.0)
            for b in [bb for _ in range(reps) for bb in range(_BPC)]:
                # fp16 column accumulators (values are -512*d, max |.| < 40k)
                cols = []
                if mins_on:
                    for h in range(2):
                        t = acc.tile(
                            [128, _HALF], f16, tag=f"col{b}_{h}", name=f"col{b}_{h}"
                        )
                        nc.gpsimd.memset(t[:], -65504.0)
                        cols.append(t)
                for p in range(_PT):
                    psAB = [
                        ps.tile([128, _HALF], f32, tag=f"ps{h}", name=f"ps{h}")
                        for h in range(2)
                    ]
                    lp = b * _N + p * 128
                    lhsT = pa[:, lp : lp + 128]
                    for h in range(2):
                        for q in range(4):
                            c0 = b * _M + h * _HALF + q * 512
                            nc.tensor.matmul(
                                psAB[h][:, q * 512 : (q + 1) * 512],
                                lhsT,
                                ga[:, c0 : c0 + 512],
                                start=True,
                                stop=True,
                            )
                    col = b * _PT + p
                    if not act_copy_on:
                        # bisect variant: VectorE reduces straight from PSUM
                        r12 = work.tile([128, 2], f32, tag="r12")
                        nc.vector.reduce_max(r12[:, 0:1], psAB[0][:], axis=X)
                        nc.vector.reduce_max(r12[:, 1:2], psAB[1][:], axis=X)
                        nc.vector.reduce_max(rowm[:, col : col + 1], r12[:], axis=X)
                        continue
                    # evacuate PSUM via ScalarE, casting to fp16 (the only
                    # rounding in the min stage; ~7e-6 on the final mean)
                    dcopy = dcp.tile([128, 2 * _HALF], f16, tag="dcopy")
                    act_copy(dcopy[:, 0:_HALF], psAB[0][:])
                    act_copy(dcopy[:, _HALF : 2 * _HALF], psAB[1][:])
                    if not mins_on:
                        continue
                    # dist1: single tensor_scalar with a fused max-accumulator
                    # (out is a throwaway copy; accum_out = row max), fp16
                    # SBUF single-src -> DVE 4x perf mode
                    dummy = work.tile([128, 2 * _HALF], f16, tag="dummy")
                    nc.vector.tensor_scalar(
                        dummy[:],
                        dcopy[:],
                        -65504.0,
                        None,
                        op0=Alu.max,
                        op1=Alu.max,
                        accum_out=rowm[:, col : col + 1],
                    )
                    # dist2: per-gt-column accumulate (fp16 SBUF = DVE 2x mode)
                    for h in range(2):
                        half = dcopy[:, h * _HALF : (h + 1) * _HALF]
                        nc.vector.tensor_tensor(
                            cols[h][:], half, cols[h][:], op=Alu.max
                        )
                if mins_on:
                    for h in range(2):
                        r = b * 2 + h
                        if par:
                            colred = work.tile([128, _HALF], f32, tag="colred")
                            nc.gpsimd.partition_all_reduce(
                                colred[:], cols[h][:], 128, bass_isa.ReduceOp.max
                            )
                            nc.sync.dma_start(colm_d[r : r + 1, :], colred[0:1, :])
                        else:
                            nc.sync.dma_start(
                                colm_d[r * 128 : (r + 1) * 128, :], cols[h][:]
                            )
            nc.sync.dma_start(rowm_d[:], rowm[:])
    nc.compile()
    return nc


def _get_runtime():
    """Build the Bass program once and wrap it in a cached sharded jit
    (mirrors bass2jax.run_bass_via_pjrt's multi-core branch so repeated
    kernel() calls reuse the compiled NEFF)."""
    global _cache
    if _cache is not None:
        return _cache

    import jax
    from jax.experimental.shard_map import shard_map
    from jax.sharding import Mesh, PartitionSpec
    import concourse.mybir as mybir
    from concourse import bass2jax

    nc = _build_nc()
    bass2jax.install_neuronx_cc_hook()

    partition_name = nc.partition_id_tensor.name if nc.partition_id_tensor else None
    in_names, out_names, out_avals = [], [], []
    for alloc in nc.m.functions[0].allocations:
        if not isinstance(alloc, mybir.MemoryLocationSet):
            continue
        name = alloc.memorylocations[0].name
        if alloc.kind == "ExternalInput":
            if name != partition_name:
                in_names.append(name)
        elif alloc.kind == "ExternalOutput":
            out_names.append(name)
            out_avals.append(
                jax.core.ShapedArray(
                    tuple(alloc.tensor_shape), mybir.dt.np(alloc.dtype)
                )
            )
    n_params = len(in_names)
    n_outs = len(out_avals)
    all_in_names = list(in_names) + list(out_names)
    if partition_name is not None:
        all_in_names.append(partition_name)

    def _body(*args):
        operands = list(args)
        if partition_name is not None:
            operands.append(bass2jax.partition_id_tensor())
        outs = bass2jax._bass_exec_p.bind(
            *operands,
            out_avals=tuple(out_avals),
            in_names=tuple(all_in_names),
            out_names=tuple(out_names),
            lowering_input_output_aliases=(),
            sim_require_finite=True,
            sim_require_nnan=True,
            nc=nc,
        )
        return tuple(outs)

    devices = jax.devices()[:_NCORES]
    assert len(devices) == _NCORES, f"need {_NCORES} cores, got {len(jax.devices())}"
    mesh = Mesh(np.asarray(devices), ("core",))
    in_specs = (PartitionSpec("core"),) * (n_params + n_outs)
    out_specs = (PartitionSpec("core"),) * n_outs
    donate = tuple(range(n_params, n_params + n_outs))
    sharded = jax.jit(
        shard_map(
            _body, mesh=mesh, in_specs=in_specs, out_specs=out_specs, check_rep=False
        ),
        donate_argnums=donate,
        keep_unused=True,
    )
    _cache = (sharded, in_names, out_names, out_avals)
    return _cache


def _split3(x):
    """fp32 -> 3 bf16 levels whose sum reproduces x to ~2^-27 relative."""
    import ml_dtypes

    bf = ml_dtypes.bfloat16
    x0 = x.astype(bf)
    r = x - x0.astype(np.float32)
    x1 = r.astype(bf)
    r -= x1.astype(np.float32)
    x2 = r.astype(bf)
    return x0, x1, x2


def _augment(prediction, gt):
    """Host-side prep: bf16 split-augmented matrices [B, 24, N]/[B, 24, M].

    (lhsT.T @ rhs)[i, j] = 2 p.g - |p|^2 - |g|^2 = -d[i, j]
    """
    import ml_dtypes

    bf = ml_dtypes.bfloat16
    pred = np.asarray(prediction, dtype=np.float32)
    g = np.asarray(gt, dtype=np.float32)
    p2 = np.sum(pred * pred, axis=-1)  # [B, N]
    g2 = np.sum(g * g, axis=-1)  # [B, M]

    predA = np.empty((_B, _K, _N), bf)
    gtA = np.empty((_B, _K, _M), bf)
    for d in range(3):
        pd0, pd1, pd2 = _split3(pred[:, :, d])
        Gd0, Gd1, Gd2 = _split3(2.0 * g[:, :, d])
        base = d * 6
        # product pairs (0,0),(0,1),(1,0),(1,1),(0,2),(2,0)
        for r, (pi, gi) in enumerate(
            [(0, 0), (0, 1), (1, 0), (1, 1), (0, 2), (2, 0)]
        ):
            predA[:, base + r, :] = (pd0, pd1, pd2)[pi]
            gtA[:, base + r, :] = (Gd0, Gd1, Gd2)[gi]
    q0, q1, q2 = _split3(p2)
    r0, r1, r2 = _split3(g2)
    for lvl, q in enumerate((q0, q1, q2)):
        predA[:, 18 + lvl, :] = q
        gtA[:, 18 + lvl, :] = bf(-1.0)
    for lvl, r in enumerate((r0, r1, r2)):
        predA[:, 21 + lvl, :] = bf(1.0)
        gtA[:, 21 + lvl, :] = -r
    # scale the product by 2^9 (16 * 32, exact in bf16) so the fp16 min
    # stage stays far from subnormals: device values are -512*d
    predA = (predA.astype(np.float32) * 16.0).astype(bf)
    gtA = (gtA.astype(np.float32) * 32.0).astype(bf)
    return predA, gtA


def kernel(prediction, gt):
    sharded, in_names, out_names, out_avals = _get_runtime()

    predA, gtA = _augment(prediction, gt)
    # per-core inputs: batches [c*BPC, (c+1)*BPC) concatenated column-wise
    per_core = {
        "predA": [
            predA[c * _BPC : (c + 1) * _BPC]
            .transpose(1, 0, 2)
            .reshape(_K, _BPC * _N)
            for c in range(_NCORES)
        ],
        "gtA": [
            gtA[c * _BPC : (c + 1) * _BPC].transpose(1, 0, 2).reshape(_K, _BPC * _M)
            for c in range(_NCORES)
        ],
    }
    concat_in = [
        np.ascontiguousarray(np.concatenate(per_core[name], axis=0))
        for name in in_names
    ]
    concat_zeros = [
        np.zeros((_NCORES * a.shape[0],) + tuple(a.shape[1:]), a.dtype)
        for a in out_avals
    ]
    out_arrs = sharded(*concat_in, *concat_zeros)

    total = 0.0
    for i, name in enumerate(out_names):
        arr = np.asarray(out_arrs[i])  # stacked along axis 0 across cores
        total += np.sum(arr, dtype=np.float64)
    # entries hold -512*dist values; dist1 has B*N entries, dist2 has B*M
    result = -total / (512.0 * float(_B * _N))
    return np.float32(result)

